# revision 1
# baseline (speedup 1.0000x reference)
"""Trainium2 Bass kernel for nn_Critic GNN message-passing critic.

Problem (hardcoded shapes): B=1024 graphs x 64 nodes x 4 feats, 1024 edges/graph
(same topology per graph), EdgeConv MLP 10->32->32, scatter-add by src, then a
per-edge critic head 73->32->1 summed over 1027 rows per graph.

Strategy: data-parallel over graphs, 128 graphs per NeuronCore x 8 cores.
All gathers/scatters become matmuls against one-hot matrices built on the host
from the runtime index tensors. W2 is folded through the segment-sum
(segment_sum(relu(.) @ W2) == segment_sum(relu(.)) @ W2), so the second MLP
layer collapses into the phase-2 node tables.
"""

import numpy as np
from contextlib import ExitStack

from concourse import bass, bacc, tile, bass_utils
from concourse import mybir

f32 = mybir.dt.float32
RELU = mybir.ActivationFunctionType.Relu
MAX = mybir.AluOpType.max
MULT = mybir.AluOpType.mult
ADD = mybir.AluOpType.add

# ---- problem constants ----
B, NN, NODE, EDGEF, HID, NFACT, NE = 1024, 64, 4, 2, 32, 3, 1024
NCORES = 8
GPC = B // NCORES          # 128 graphs per core
NTG = GPC // 16            # 8 groups of 16 graphs
NSG = GPC // 4             # 32 subgroups of 4 graphs
EC = NE // 128             # 8 edge chunks of 128
E2 = NE + NFACT            # 1027
E2P = 1152                 # padded to 9*128
SPLIT2 = 576               # phase-2 relu/accum column split (ACT|DVE)

_CACHE = {}


def _build_nc(debug=False):
    nc = bacc.Bacc("TRN2", target_bir_lowering=False, debug=False,
                   num_devices=NCORES)

    def din(name, shape):
        return nc.dram_tensor(name, shape, f32, kind="ExternalInput").ap()

    # per-core data
    xT = din("xT", [64, 64 * NTG])          # [(16g,4f), n] per 16-graph group
    xT2 = din("xT2", [16, 64 * NSG])        # [(4g,4f), n] per subgroup (base-0)
    eaT = din("eaT", [33, 128 * NTG * EC])  # [(16g,2c)+ones, e] tiles
    Gt = din("Gt", [128, NE])               # one-hot gather (src|dst) columns=e
    St = din("St", [128, 64 * EC])          # one-hot scatter chunks
    G2t = din("G2t", [128, E2P])            # phase-2 gather, zero-padded cols
    actB = din("actB", [96, 2 * E2P])       # banded action rows (base 0/32/64)
    selP = din("selP", [96, 128 * 8])       # banded wl_c selectors
    blcol = din("blcol", [128, 1])          # bl[j] per (g,j) partition
    # constants (same on all cores)
    W1a_blk = din("W1a_blk", [64, 512])
    W1b_blk = din("W1b_blk", [64, 512])
    W1cb = din("W1cb", [33, 512])
    Wla4_blk = din("Wla4_blk", [16, 128])
    Wlap_blk = din("Wlap_blk", [128, 128])
    Wlb4_blk = din("Wlb4_blk", [16, 128])
    Wlbp_blk = din("Wlbp_blk", [128, 128])
    V2corr = din("V2corr", [128, 128])      # c_n * b2-fold correction (zeros if b2==0)
    ident = din("ident", [64, 64])
    WvP = din("WvP", [128, 4])
    vout = nc.dram_tensor("v", [4, 2 * NSG], f32, kind="ExternalOutput").ap()
    dbg = {}
    if debug:
        for name, shape in [("dbg_V1", [128, 512 * NTG]), ("dbg_U", [64, 512 * NTG]),
                            ("dbg_UT", [128, 64 * NSG]), ("dbg_V2", [128, 128 * NSG]),
                            ("dbg_S1", [128, 2 * NSG])]:
            dbg[name] = nc.dram_tensor(name, shape, f32, kind="ExternalOutput").ap()

    with tile.TileContext(nc) as tc:
        with ExitStack() as ctx:
            cpool = ctx.enter_context(tc.tile_pool(name="consts", bufs=1))

            def load(ap, shape, tag):
                t = cpool.tile(shape, f32, tag=tag)
                nc.sync.dma_start(t[:], ap[:])
                return t

            t_xT = load(xT, [64, 64 * NTG], "xT")
            t_xT2 = load(xT2, [16, 64 * NSG], "xT2")
            t_eaT = load(eaT, [33, 128 * NTG * EC], "eaT")
            t_Gt = load(Gt, [128, NE], "Gt")
            t_St = load(St, [128, 64 * EC], "St")
            t_G2t = load(G2t, [128, E2P], "G2t")
            t_actB = load(actB, [96, 2 * E2P], "actB")
            t_selP = load(selP, [96, 128 * 8], "selP")
            t_blc = load(blcol, [128, 1], "blcol")
            t_W1a = load(W1a_blk, [64, 512], "W1a")
            t_W1b = load(W1b_blk, [64, 512], "W1b")
            t_W1cb = load(W1cb, [33, 512], "W1cb")
            t_Wla4 = load(Wla4_blk, [16, 128], "Wla4")
            t_Wlap = load(Wlap_blk, [128, 128], "Wlap")
            t_Wlb4 = load(Wlb4_blk, [16, 128], "Wlb4")
            t_Wlbp = load(Wlbp_blk, [128, 128], "Wlbp")
            t_V2c = load(V2corr, [128, 128], "V2c")
            t_id = load(ident, [64, 64], "ident")
            t_WvP = load(WvP, [128, 4], "WvP")

            # persistent SBUF intermediates
            t_V1 = cpool.tile([128, 512 * NTG], f32, tag="V1")     # [slots,(16g,32j)]
            t_U = cpool.tile([64, 512 * NTG], f32, tag="U")        # [n,(16g,32j)]
            t_UT = cpool.tile([128, 64 * NSG], f32, tag="UT")      # [(4g,32jj), n]
            t_V2 = cpool.tile([128, 128 * NSG], f32, tag="V2")     # [slots,(4g,32j)]
            t_S1 = cpool.tile([128, 2 * NSG], f32, tag="S1")       # relu-sum accums

            # ---------------- phase A: V1 = [x@W1a ; x@W1b] ----------------
            with tc.tile_pool(name="psA", bufs=2, space=bass.MemorySpace.PSUM) as psA:
                for tg in range(NTG):
                    pv = psA.tile([128, 512], f32, tag="pv")
                    lx = t_xT[:, tg * 64:(tg + 1) * 64]
                    nc.tensor.matmul(pv[0:64, :], lx, t_W1a[:], start=True, stop=True)
                    nc.tensor.matmul(pv[64:128, :], lx, t_W1b[:], start=True, stop=True)
                    dst = t_V1[:, tg * 512:(tg + 1) * 512]
                    nc.scalar.copy(dst[:, 0:256], pv[:, 0:256])
                    nc.vector.tensor_copy(dst[:, 256:512], pv[:, 256:512])

            # ---------------- phase B: pre1 -> relu -> U ----------------
            with tc.tile_pool(name="psB", bufs=3, space=bass.MemorySpace.PSUM) as psB, \
                 tc.tile_pool(name="psU", bufs=2, space=bass.MemorySpace.PSUM) as psU, \
                 tc.tile_pool(name="relu1", bufs=4) as rpool:
                for tg in range(NTG):
                    pu = psU.tile([64, 512], f32, tag="pu")
                    for c in range(EC):
                        p1 = psB.tile([128, 512], f32, tag="p1")
                        gt = t_Gt[:, c * 128:(c + 1) * 128]
                        v1 = t_V1[:, tg * 512:(tg + 1) * 512]
                        nc.tensor.matmul(p1[:], gt, v1, start=True, stop=False)
                        ea = t_eaT[:, (tg * EC + c) * 128:(tg * EC + c + 1) * 128]
                        nc.tensor.matmul(p1[:], ea, t_W1cb[:], start=False, stop=True)
                        r1 = rpool.tile([128, 512], f32, tag="r1")
                        nc.scalar.activation(r1[:, 0:256], p1[:, 0:256], RELU)
                        nc.vector.tensor_scalar_max(r1[:, 256:512], p1[:, 256:512], 0.0)
                        st = t_St[:, c * 64:(c + 1) * 64]
                        nc.tensor.matmul(pu[:], st, r1[:],
                                         start=(c == 0), stop=(c == EC - 1))
                    dst = t_U[:, tg * 512:(tg + 1) * 512]
                    nc.scalar.copy(dst[:, 0:256], pu[:, 0:256])
                    nc.vector.tensor_copy(dst[:, 256:512], pu[:, 256:512])

            # ---------------- phase C: U^T, V2 tables ----------------
            with tc.tile_pool(name="psT", bufs=2, space=bass.MemorySpace.PSUM) as psT, \
                 tc.tile_pool(name="psV2", bufs=2, space=bass.MemorySpace.PSUM) as psV2:
                for tg in range(NTG):
                    pt = psT.tile([128, 256], f32, tag="pt")
                    for sl in range(4):
                        blk = t_U[:, tg * 512 + sl * 128: tg * 512 + (sl + 1) * 128]
                        nc.tensor.transpose(pt[:, sl * 64:(sl + 1) * 64], blk, t_id[:])
                    dst = t_UT[:, tg * 256:(tg + 1) * 256]
                    nc.scalar.copy(dst[:, 0:128], pt[:, 0:128])
                    nc.vector.tensor_copy(dst[:, 128:256], pt[:, 128:256])
                for sg in range(NSG):
                    tg, sl = sg // 4, sg % 4
                    pv2 = psV2.tile([128, 128], f32, tag="pv2")
                    lx = t_xT2[:, sg * 64:(sg + 1) * 64]
                    ut = t_UT[:, sg * 64:(sg + 1) * 64]
                    nc.tensor.matmul(pv2[0:64, :], lx, t_Wla4[:], start=True, stop=False)
                    nc.tensor.matmul(pv2[0:64, :], ut, t_Wlap[:], start=False, stop=True)
                    nc.tensor.matmul(pv2[64:128, :], lx, t_Wlb4[:], start=True, stop=False)
                    nc.tensor.matmul(pv2[64:128, :], ut, t_Wlbp[:], start=False, stop=True)
                    dst = t_V2[:, sg * 128:(sg + 1) * 128]
                    # add the c_n * b2 fold while evacuating
                    nc.vector.scalar_tensor_tensor(
                        dst[:, 0:64], pv2[:, 0:64], 1.0,
                        t_V2c[:, 0:64], MULT, ADD)
                    nc.vector.scalar_tensor_tensor(
                        dst[:, 64:128], pv2[:, 64:128], 1.0,
                        t_V2c[:, 64:128], MULT, ADD)

            # ---------------- phase D: pre2 -> relu-sum ----------------
            with tc.tile_pool(name="psD", bufs=2, space=bass.MemorySpace.PSUM) as psD, \
                 tc.tile_pool(name="scr2", bufs=2) as spool:
                t_z = spool.tile([128, E2P - SPLIT2], f32, tag="zeros")
                nc.gpsimd.memset(t_z[:], 0.0)
                nsplits = [(0, 512), (512, 1024), (1024, E2P)]
                for sg in range(NSG):
                    slot = 1 if sg >= 24 else 0
                    band = (sg // 8) % 3 if slot == 0 else 0
                    p = sg % 8 if slot == 0 else sg - 24
                    p2 = psD.tile([128, E2P], f32, tag="p2")
                    v2 = t_V2[:, sg * 128:(sg + 1) * 128]
                    sel = t_selP[band * 32:(band + 1) * 32, p * 128:(p + 1) * 128]
                    for (a, b) in nsplits:
                        nc.tensor.matmul(p2[:, a:b], v2, t_G2t[:, a:b],
                                         start=True, stop=False)
                        arows = t_actB[band * 32:(band + 1) * 32,
                                       slot * E2P + a: slot * E2P + b]
                        nc.tensor.matmul(p2[:, a:b], sel, arows,
                                         start=False, stop=True)
                    scr = spool.tile([128, E2P], f32, tag="scr")
                    nc.scalar.activation(scr[:, 0:SPLIT2], p2[:, 0:SPLIT2], RELU,
                                         bias=t_blc[:],
                                         accum_out=t_S1[:, 2 * sg:2 * sg + 1])
                    nc.vector.scalar_tensor_tensor(
                        scr[:, SPLIT2:E2P], p2[:, SPLIT2:E2P], t_blc[:], t_z[:],
                        ADD, MAX, accum_out=t_S1[:, 2 * sg + 1:2 * sg + 2])

            if debug:
                for name, t in [("dbg_V1", t_V1), ("dbg_U", t_U), ("dbg_UT", t_UT),
                                ("dbg_V2", t_V2), ("dbg_S1", t_S1)]:
                    nc.sync.dma_start(dbg[name][:], t[:])

            # ---------------- finale: fold Wv ----------------
            with tc.tile_pool(name="psF", bufs=1, space=bass.MemorySpace.PSUM) as psF, \
                 tc.tile_pool(name="fin", bufs=1) as fpool:
                pf = psF.tile([4, 2 * NSG], f32, tag="pf")
                nc.tensor.matmul(pf[:], t_WvP[:], t_S1[:], start=True, stop=True)
                fo = fpool.tile([4, 2 * NSG], f32, tag="fo")
                nc.vector.tensor_copy(fo[:], pf[:])
                nc.sync.dma_start(vout[:], fo[:])

    nc.compile()
    return nc


def _blkdiag(blocks_w, g_count, rows_per_g, cols_per_g, W):
    """out[(g,rows), (g,cols)] = W  block-diagonal replication."""
    out = np.zeros((g_count * rows_per_g, g_count * cols_per_g), np.float32)
    for g in range(g_count):
        out[g * rows_per_g:(g + 1) * rows_per_g,
            g * cols_per_g:(g + 1) * cols_per_g] = W
    return out


def _host_prep(inputs):
    x = np.ascontiguousarray(np.asarray(inputs["x"], np.float32))
    ea = np.ascontiguousarray(np.asarray(inputs["edge_attr"], np.float32))
    act = np.ascontiguousarray(np.asarray(inputs["action"], np.float32))
    es = np.asarray(inputs["edges_src"]).astype(np.int64)
    ed = np.asarray(inputs["edges_dst"]).astype(np.int64)
    W1 = np.asarray(inputs["W1"], np.float32)
    b1 = np.asarray(inputs["b1"], np.float32)
    W2 = np.asarray(inputs["W2"], np.float32)
    b2 = np.asarray(inputs["b2"], np.float32)
    Wl = np.asarray(inputs["Wl"], np.float32)
    bl = np.asarray(inputs["bl"], np.float32)
    Wv = np.asarray(inputs["Wv"], np.float32)
    bv = np.asarray(inputs["bv"], np.float32)

    W1a, W1b, W1c = W1[0:4], W1[4:8], W1[8:10]
    Wla4 = Wl[0:4]
    Wlap = W2 @ Wl[4:36]       # fold W2 into phase-2 src table
    Wlb4 = Wl[36:40]
    Wlbp = W2 @ Wl[40:72]
    wlc = Wl[72]               # [32]

    consts = {}
    consts["W1a_blk"] = _blkdiag(None, 16, 4, 32, W1a)
    consts["W1b_blk"] = _blkdiag(None, 16, 4, 32, W1b)
    w1cb = np.zeros((33, 512), np.float32)
    for g in range(16):
        w1cb[2 * g:2 * g + 2, 32 * g:32 * g + 32] = W1c
        w1cb[32, 32 * g:32 * g + 32] = b1
    consts["W1cb"] = w1cb
    consts["Wla4_blk"] = _blkdiag(None, 4, 4, 32, Wla4)
    consts["Wlap_blk"] = _blkdiag(None, 4, 32, 32, Wlap)
    consts["Wlb4_blk"] = _blkdiag(None, 4, 4, 32, Wlb4)
    consts["Wlbp_blk"] = _blkdiag(None, 4, 32, 32, Wlbp)
    # banded wl_c selectors: for each band (replicated at bases 0/32/64) and
    # position p in band, select the 4 action rows of that subgroup
    selp = np.zeros((96, 128 * 8), np.float32)
    for band in range(3):
        for p in range(8):
            for g in range(4):
                selp[band * 32 + 4 * p + g, p * 128 + 32 * g:p * 128 + 32 * g + 32] = wlc
    consts["selP"] = selp
    blcol = np.zeros((128, 1), np.float32)
    for g in range(4):
        blcol[32 * g:32 * g + 32, 0] = bl
    consts["blcol"] = blcol
    consts["ident"] = np.eye(64, dtype=np.float32)
    wvp = np.zeros((128, 4), np.float32)
    for g in range(4):
        wvp[32 * g:32 * g + 32, g] = Wv[:, 0]
    consts["WvP"] = wvp

    # one-hot gather/scatter matrices (shared topology across graphs)
    gt = np.zeros((128, NE), np.float32)
    gt[es, np.arange(NE)] = 1.0
    gt[64 + ed, np.arange(NE)] += 1.0
    consts["Gt"] = gt
    st = np.zeros((128, 64 * EC), np.float32)
    for c in range(EC):
        st[np.arange(128), c * 64 + es[c * 128:(c + 1) * 128]] = 1.0
    consts["St"] = st
    g2t = np.zeros((128, E2P), np.float32)
    g2t[:, :NE] = gt
    for i in range(NFACT):
        g2t[61 + i, NE + i] = 1.0
        g2t[64 + 61 + i, NE + i] += 1.0
    consts["G2t"] = g2t

    # c_n * b2 correction folded into V2 (x_pp = U@W2 + c_n*b2)
    cn = np.bincount(es, minlength=64).astype(np.float32)  # [64]
    v2c = np.zeros((128, 128), np.float32)
    corr_a = np.outer(cn, b2 @ Wl[4:36])   # [64, 32]
    corr_b = np.outer(cn, b2 @ Wl[40:72])
    for g in range(4):
        v2c[0:64, 32 * g:32 * g + 32] = corr_a
        v2c[64:128, 32 * g:32 * g + 32] = corr_b
    consts["V2corr"] = v2c

    x3 = x.reshape(B, NN, NODE)
    ea4 = ea.reshape(B, NE, EDGEF)
    in_maps = []
    for t in range(NCORES):
        m = dict(consts)
        xs = x3[t * GPC:(t + 1) * GPC]          # [128, 64, 4]
        xT = np.zeros((64, 64 * NTG), np.float32)
        for tg in range(NTG):
            blk = xs[tg * 16:(tg + 1) * 16]     # [16, 64, 4]
            xT[:, tg * 64:(tg + 1) * 64] = blk.transpose(0, 2, 1).reshape(64, 64)
        m["xT"] = xT
        xT2 = np.zeros((16, 64 * NSG), np.float32)
        for sg in range(NSG):
            blk = xs[4 * sg:4 * sg + 4]     # [4, 64, 4]
            xT2[:, sg * 64:(sg + 1) * 64] = blk.transpose(0, 2, 1).reshape(16, 64)
        m["xT2"] = xT2
        eas = ea4[t * GPC:(t + 1) * GPC]        # [128, 1024, 2]
        eaT = np.ones((33, 128 * NTG * EC), np.float32)
        for tg in range(NTG):
            for c in range(EC):
                blk = eas[tg * 16:(tg + 1) * 16, c * 128:(c + 1) * 128]  # [16,128,2]
                col = (tg * EC + c) * 128
                eaT[0:32, col:col + 128] = blk.transpose(0, 2, 1).reshape(32, 128)
        m["eaT"] = eaT
        acs = act[t * GPC:(t + 1) * GPC]        # [128, 1027]
        blob = np.zeros((96, 2 * E2P), np.float32)
        for sg in range(NSG):
            slot = 1 if sg >= 24 else 0
            band = (sg // 8) % 3 if slot == 0 else 0
            p = sg % 8 if slot == 0 else sg - 24
            blob[band * 32 + 4 * p:band * 32 + 4 * p + 4,
                 slot * E2P:slot * E2P + E2] = acs[4 * sg:4 * sg + 4]
        m["actB"] = blob
        in_maps.append(m)
    # 1027*bv plus correction for the 125 padded columns that get relu(bl)
    pad_bias = (E2P - E2) * float(np.maximum(bl, 0.0) @ Wv[:, 0])
    extra = float(E2) * float(bv.reshape(-1)[0]) - pad_bias
    return in_maps, extra


def kernel(**inputs) -> np.ndarray:
    if "nc" not in _CACHE:
        _CACHE["nc"] = _build_nc()
    nc = _CACHE["nc"]
    in_maps, extra = _host_prep(inputs)
    res = bass_utils.run_bass_kernel_spmd(nc, in_maps, list(range(NCORES)))
    out = np.empty((B,), np.float32)
    for t in range(NCORES):
        v = res.results[t]["v"]                 # [4, 2*NSG]
        per = v[:, 0::2] + v[:, 1::2]           # [4, NSG]
        out[t * GPC:(t + 1) * GPC] = per.T.reshape(-1) + extra
    return out



# revision 3
# speedup vs baseline: 21667.2379x; 21667.2379x over previous
"""Trainium2 Bass kernel v2 for nn_Critic GNN message-passing critic.

Data-parallel: 128 graphs/core x 8 cores. All PE operands bf16 (fp32 matmul
costs 4 cyc/row on TRN2; bf16 costs 1). Per-edge endpoint features are
gathered on the HOST into gxE (x_src|x_dst|edge_attr|ones rows), so the edge
MLP's first layer is ONE 81-row stationary matmul per (8-graph group, 128-edge
chunk). W2 is folded through the scatter-add. The critic head runs per
4-graph subgroup: 1024 edge slots in a [128,1024] PSUM tile (512+512 relu
splits ACT/DVE) and the 3 factory slots in a shared per-pair tile feeding a
second finale matmul, so there is no padding anywhere.

PSUM budget (8 banks): tag A [128,512]f32 x3 + T [128,256]bf16 x1 +
P6 [128,1024]f32 x2  = 3 + 1 + 4.
"""

import numpy as np
import ml_dtypes
from contextlib import ExitStack

from concourse import bass, bacc, tile, bass_utils
from concourse import mybir

f32 = mybir.dt.float32
bf16 = mybir.dt.bfloat16
BF = ml_dtypes.bfloat16
RELU = mybir.ActivationFunctionType.Relu
MAX = mybir.AluOpType.max
ADD = mybir.AluOpType.add

B, NN, NODE, EDGEF, HID, NFACT, NE = 1024, 64, 4, 2, 32, 3, 1024
NCORES = 8
GPC = B // NCORES          # 128 graphs per core
NPAIR = 8                  # pairs of 8-graph groups
E2 = NE + NFACT            # 1027

_CACHE = {}


def _build_nc(nrep=1, no_fact=False, no_strip32=False):
    nc = bacc.Bacc("TRN2", target_bir_lowering=False, debug=False,
                   num_devices=NCORES)

    def din(name, shape, dt=bf16):
        return nc.dram_tensor(name, shape, dt, kind="ExternalInput").ap()

    gxE = din("gxE", [81, 16 * NE])        # gathered x|ea|ones per 8g-group
    Wblk = din("Wblk", [81, 256])
    St = din("St", [128, 8 * 64])
    idB = din("idB", [128, 64])
    xT2c = din("xT2c", [17, 32 * 64])
    Wla4c = din("Wla4c", [17, 128])
    Wlb4c = din("Wlb4c", [17, 128])
    WlapB = din("WlapB", [128, 128])
    WlbpB = din("WlbpB", [128, 128])
    G2t = din("G2t", [128, E2 + 13])
    selQ = din("selQ", [128, 128])
    selC8 = din("selC8", [8, 128])
    actH0 = din("actH0", [8, 16 * E2])
    actH1 = din("actH1", [4, 16 * E2])
    blc = din("blc", [128, 1], f32)
    WvP = din("WvP", [128, 4], f32)
    WvPb = din("WvPb", [128, 4])
    vout = nc.dram_tensor("v", [4, 160], f32, kind="ExternalOutput").ap()

    with tile.TileContext(nc) as tc:
        with ExitStack() as ctx:
            cpool = ctx.enter_context(tc.tile_pool(name="consts", bufs=1))
            rpool = ctx.enter_context(tc.tile_pool(name="r1p", bufs=10))
            upool = ctx.enter_context(tc.tile_pool(name="uu", bufs=2))
            utpool = ctx.enter_context(tc.tile_pool(name="utp", bufs=2))
            v2pool = ctx.enter_context(tc.tile_pool(name="v2p", bufs=3))
            spool = ctx.enter_context(tc.tile_pool(name="scr", bufs=2))
            psA = ctx.enter_context(
                tc.tile_pool(name="psA", bufs=7, space=bass.MemorySpace.PSUM))
            psT = ctx.enter_context(
                tc.tile_pool(name="psT", bufs=1, space=bass.MemorySpace.PSUM))

            for _rep in range(nrep):
                def load(ap, shape, tag, dt=bf16):
                    t = cpool.tile(shape, dt, tag=tag, name=tag)
                    nc.sync.dma_start(t[:], ap[:])
                    return t

                t_gxE = load(gxE, [81, 16 * NE], "gxE")
                t_Wblk = load(Wblk, [81, 256], "Wblk")
                t_St = load(St, [128, 8 * 64], "St")
                t_idB = load(idB, [128, 64], "idB")
                t_xT2c = load(xT2c, [17, 32 * 64], "xT2c")
                t_Wla4c = load(Wla4c, [17, 128], "Wla4c")
                t_Wlb4c = load(Wlb4c, [17, 128], "Wlb4c")
                t_WlapB = load(WlapB, [128, 128], "WlapB")
                t_WlbpB = load(WlbpB, [128, 128], "WlbpB")
                t_G2t = load(G2t, [128, E2 + 13], "G2t")
                t_selQ = load(selQ, [128, 128], "selQ")
                t_selC8 = load(selC8, [8, 128], "selC8")
                t_blc = load(blc, [128, 1], "blc", f32)
                t_WvP = load(WvP, [128, 4], "WvP", f32)
                t_WvPb = load(WvPb, [128, 4], "WvPb")
                t_asb0 = load(actH0, [8, 16 * E2], "asb0")
                t_asb1 = cpool.tile([36, 16 * E2], bf16, tag="asb1", name="asb1")
                nc.sync.dma_start(t_asb1[32:36, :], actH1[:])
                t_S1 = cpool.tile([128, 64], f32, tag="S1", name="S1")
                t_z = cpool.tile([128, 512], f32, tag="zz", name="zz")
                nc.gpsimd.memset(t_z[:], 0.0)
                t_factS = cpool.tile([128, 96], bf16, tag="factS", name="factS")
                nc.gpsimd.memset(t_factS[:], 0.0)

                def ph2(p):
                    """Edge MLP layer1 + relu for both 8g-groups of pair p."""
                    r1s = {0: [], 1: []}
                    for cc in range(4):
                        for gi, g8 in ((0, 2 * p), (1, 2 * p + 1)):
                            p1 = psA.tile([128, 512], f32, tag="A", name="p1")
                            for h in range(2):
                                c = 2 * cc + h
                                off = g8 * NE + c * 128
                                nc.tensor.matmul(
                                    p1[:, h * 256:(h + 1) * 256],
                                    t_gxE[:, off:off + 128], t_Wblk[:],
                                    start=True, stop=True)
                            r = rpool.tile([128, 512], bf16, tag="r1", name="r1")
                            if (cc + gi) % 2 == 0:
                                nc.scalar.activation(r[:], p1[:], RELU)
                            else:
                                nc.vector.tensor_scalar_max(r[:], p1[:], 0.0)
                            r1s[gi].append(r)
                    return r1s

                def ph3(p, r1s):
                    """Scatter burst + U evacuation."""
                    pu = psA.tile([128, 512], f32, tag="A", name="pu")
                    for c in range(8):
                        stc = t_St[:, c * 64:(c + 1) * 64]
                        sl = slice((c % 2) * 256, (c % 2) * 256 + 256)
                        nc.tensor.matmul(pu[0:64, 0:256], stc, r1s[0][c // 2][:, sl],
                                         start=(c == 0), stop=(c == 7))
                        # sim's group check mis-addresses partition-base-64
                        # groups; hardware has_written is per element
                        nc.tensor.matmul(pu[64:128, 0:256], stc, r1s[1][c // 2][:, sl],
                                         start=(c == 0), stop=(c == 7),
                                         skip_group_check=True)
                    # two base-0 tiles: PE transpose hangs on inputs at
                    # partition base 64, so evacuate each group to base 0
                    t_UA = upool.tile([64, 256], bf16, tag="UA", name="UA")
                    t_UB = upool.tile([64, 256], bf16, tag="UB", name="UB")
                    if p % 2 == 0:
                        nc.scalar.copy(t_UA[0:64, :], pu[0:64, 0:256])
                        nc.vector.tensor_copy(t_UB[0:64, :], pu[64:128, 0:256])
                    else:
                        nc.vector.tensor_copy(t_UA[0:64, :], pu[0:64, 0:256])
                        nc.scalar.copy(t_UB[0:64, :], pu[64:128, 0:256])
                    return (t_UA, t_UB)

                def tail(p, t_U):
                    """ph4 transposes, ph5 V2 tables, ph6 critic head for pair p."""
                    t_UA, t_UB = t_U
                    pt = psT.tile([128, 256], bf16, tag="T", name="pt")
                    for k in range(4):
                        ut_src = t_UA if k < 2 else t_UB
                        src = ut_src[0:64, (k % 2) * 128:(k % 2) * 128 + 128]
                        idn = t_idB[0:64, :]
                        nc.tensor.transpose(pt[:, k * 64:(k + 1) * 64], src, idn)
                    t_UT = utpool.tile([128, 256], bf16, tag="UT", name="UT")
                    if p % 2 == 0:
                        nc.vector.tensor_copy(t_UT[:], pt[:])
                    else:
                        nc.scalar.copy(t_UT[:], pt[:])

                    v2t = []
                    for h in range(2):
                        pv = psA.tile([128, 512], f32, tag="A", name="pv")
                        for k2 in range(2):
                            k = 2 * h + k2
                            s = 4 * p + k
                            col = slice(k2 * 128, (k2 + 1) * 128)
                            lx = t_xT2c[:, s * 64:(s + 1) * 64]
                            ut = t_UT[:, k * 64:(k + 1) * 64]
                            nc.tensor.matmul(pv[0:64, col], lx, t_Wla4c[:],
                                             start=True, stop=False)
                            nc.tensor.matmul(pv[0:64, col], ut, t_WlapB[:],
                                             start=False, stop=True)
                            nc.tensor.matmul(pv[64:128, col], lx, t_Wlb4c[:],
                                             start=True, stop=False,
                                             skip_group_check=True)
                            nc.tensor.matmul(pv[64:128, col], ut, t_WlbpB[:],
                                             start=False, stop=True,
                                             skip_group_check=True)
                        v2 = v2pool.tile([128, 256], bf16, tag="v2", name="v2")
                        if h == 0:
                            nc.scalar.copy(v2[:], pv[:, 0:256])
                        else:
                            nc.vector.tensor_copy(v2[:], pv[:, 0:256])
                        v2t.append(v2)

                    factP = psA.tile([128, 512], f32, tag="A", name="factP")
                    zeroed = False
                    for h in range(2):
                        q = 2 * p + h
                        v2 = v2t[h]
                        p6s = []
                        for si in range(2):
                            fcol = slice((2 * h + si) * 3, (2 * h + si) * 3 + 3)
                            vsl = v2[:, si * 128:(si + 1) * 128]
                            pA6 = psA.tile([128, 512], f32, tag="A", name="pA6")
                            pB6 = psA.tile([128, 512], f32, tag="A", name="pB6")
                            nc.tensor.matmul(pA6[:], vsl, t_G2t[:, 0:512],
                                             start=True, stop=False)
                            nc.tensor.matmul(pB6[:], vsl, t_G2t[:, 512:1024],
                                             start=True, stop=False)
                            if no_fact:
                                p6s.append((pA6, pB6))
                                continue
                            if not zeroed:
                                # zeroing matmul opens the region and orders all
                                # factory matmuls after it via write hazards
                                nc.tensor.matmul(factP[:, 0:12], vsl,
                                                 t_G2t[:, E2 + 1:E2 + 13],
                                                 start=True, stop=False)
                                zeroed = True
                            nc.tensor.matmul(factP[:, fcol], vsl, t_G2t[:, 1024:1027],
                                             start=False, stop=False,
                                             skip_group_check=True)
                            p6s.append((pA6, pB6))
                        # action matmuls, interleaved across strips for overlap
                        for blk in range(3):
                            for si in range(2):
                                pA6, pB6 = p6s[si]
                                sel = t_selQ[32 * (0 if no_strip32 else si):
                                             32 * (0 if no_strip32 else si) + 4, :]
                                if blk == 0:
                                    dst, a0, a1 = pA6[:], 0, 512
                                elif blk == 1:
                                    dst, a0, a1 = pB6[:], 512, 1024
                                else:
                                    if no_fact:
                                        continue
                                    # factory act MMs run at strip 0 for BOTH
                                    # sgs (strip-32 small-N matmuls hang HW);
                                    # si=1 uses a K=8 sel with zero top rows
                                    fcol = slice((2 * h + si) * 3, (2 * h + si) * 3 + 3)
                                    dst = factP[:, fcol]
                                    if si == 0:
                                        selc = t_selQ[0:4, :]
                                        arow = asb = t_asb0[0:4,
                                            q * E2 + 1024:q * E2 + 1027]
                                    else:
                                        selc = t_selC8[0:8, :]
                                        arow = t_asb0[0:8,
                                            q * E2 + 1024:q * E2 + 1027]
                                    nc.tensor.matmul(dst, selc, arow,
                                                     start=False, stop=False,
                                                     skip_group_check=True)
                                    continue
                                ss = 0 if no_strip32 else si
                                asrc = t_asb0 if ss == 0 else t_asb1
                                arow = asrc[32 * ss:32 * ss + 4,
                                            q * E2 + a0:q * E2 + a1]
                                nc.tensor.matmul(dst, sel, arow,
                                                 start=False, stop=True)
                        for si in range(2):
                            s = 2 * q + si
                            pA6, pB6 = p6s[si]
                            scrA = spool.tile([128, 512], bf16, tag="scrA", name="scrA")
                            scrB = spool.tile([128, 512], bf16, tag="scrB", name="scrB")
                            nc.scalar.activation(scrA[:], pA6[:], RELU,
                                                 bias=t_blc[:],
                                                 accum_out=t_S1[:, 2 * s:2 * s + 1])
                            nc.vector.scalar_tensor_tensor(
                                scrB[:], pB6[:], t_blc[:], t_z[:],
                                ADD, MAX,
                                accum_out=t_S1[:, 2 * s + 1:2 * s + 2])
                    if not no_fact:
                        # closing no-op accumulate stops the factP group
                        nc.tensor.matmul(factP[:, 0:12], v2t[1][:, 0:128],
                                         t_G2t[:, E2 + 1:E2 + 13],
                                         start=False, stop=True)
                        # factory columns: relu once per pair into factS
                        nc.vector.tensor_scalar(
                            t_factS[:, p * 12:(p + 1) * 12], factP[:, 0:12],
                            t_blc[:], 0.0, ADD, MAX)

                # software pipeline: pair p's MLP overlaps pair p-1's tail
                prevU = None
                for p in range(NPAIR):
                    r1s = ph2(p)
                    if prevU is not None:
                        tail(p - 1, prevU)
                    prevU = ph3(p, r1s)
                tail(NPAIR - 1, prevU)

                # ---- finale ----
                pf = psA.tile([128, 512], f32, tag="A", name="pf")
                nc.tensor.matmul(pf[0:4, 0:64], t_WvP[:], t_S1[:],
                                 start=True, stop=True)
                nc.tensor.matmul(pf[0:4, 64:160], t_WvPb[:], t_factS[:],
                                 start=True, stop=True)
                fo = cpool.tile([4, 160], f32, tag="fo", name="fo")
                nc.vector.tensor_copy(fo[:], pf[0:4, 0:160])
                nc.sync.dma_start(vout[:], fo[:])

    nc.compile()
    return nc


def _host_prep(inputs):
    x = np.asarray(inputs["x"], np.float32).reshape(B, NN, NODE)
    ea = np.asarray(inputs["edge_attr"], np.float32).reshape(B, NE, EDGEF)
    act = np.asarray(inputs["action"], np.float32)
    es = np.asarray(inputs["edges_src"]).astype(np.int64)
    ed = np.asarray(inputs["edges_dst"]).astype(np.int64)
    W1 = np.asarray(inputs["W1"], np.float32)
    b1 = np.asarray(inputs["b1"], np.float32)
    W2 = np.asarray(inputs["W2"], np.float32)
    b2 = np.asarray(inputs["b2"], np.float32)
    Wl = np.asarray(inputs["Wl"], np.float32)
    bl = np.asarray(inputs["bl"], np.float32)
    Wv = np.asarray(inputs["Wv"], np.float32)
    bv = np.asarray(inputs["bv"], np.float32)

    Wlap = W2 @ Wl[4:36]
    Wlbp = W2 @ Wl[40:72]
    wlc = Wl[72]
    cn = np.bincount(es, minlength=NN).astype(np.float32)
    corr_a = b2 @ Wlap
    corr_b = b2 @ Wlbp

    consts = {}
    Wblk = np.zeros((81, 256), np.float32)
    for gl in range(8):
        sl = slice(32 * gl, 32 * gl + 32)
        Wblk[8 * gl:8 * gl + 4, sl] = W1[0:4]
        Wblk[8 * gl + 4:8 * gl + 8, sl] = W1[4:8]
        Wblk[64 + 2 * gl:64 + 2 * gl + 2, sl] = W1[8:10]
        Wblk[80, sl] = b1
    consts["Wblk"] = Wblk.astype(BF)

    St = np.zeros((128, 8 * 64), np.float32)
    for c in range(8):
        St[np.arange(128), c * 64 + es[c * 128:(c + 1) * 128]] = 1.0
    consts["St"] = St.astype(BF)

    idB = np.zeros((128, 64), np.float32)
    idB[0:64] = np.eye(64)
    idB[64:128] = np.eye(64)
    consts["idB"] = idB.astype(BF)

    Wla4c = np.zeros((17, 128), np.float32)
    Wlb4c = np.zeros((17, 128), np.float32)
    WlapB = np.zeros((128, 128), np.float32)
    WlbpB = np.zeros((128, 128), np.float32)
    for gl in range(4):
        sl = slice(32 * gl, 32 * gl + 32)
        Wla4c[4 * gl:4 * gl + 4, sl] = Wl[0:4]
        Wlb4c[4 * gl:4 * gl + 4, sl] = Wl[36:40]
        Wla4c[16, sl] = corr_a
        Wlb4c[16, sl] = corr_b
        WlapB[sl, sl] = Wlap
        WlbpB[sl, sl] = Wlbp
    consts["Wla4c"] = Wla4c.astype(BF)
    consts["Wlb4c"] = Wlb4c.astype(BF)
    consts["WlapB"] = WlapB.astype(BF)
    consts["WlbpB"] = WlbpB.astype(BF)

    G2t = np.zeros((128, E2 + 13), np.float32)
    G2t[es, np.arange(NE)] = 1.0
    G2t[64 + ed, np.arange(NE)] += 1.0
    for i in range(NFACT):
        G2t[61 + i, NE + i] = 1.0
        G2t[64 + 61 + i, NE + i] = 1.0
    consts["G2t"] = G2t.astype(BF)

    selQ = np.zeros((128, 128), np.float32)
    for i in range(4):
        for k in range(4):
            selQ[32 * i + k, 32 * k:32 * k + 32] = wlc
    consts["selQ"] = selQ.astype(BF)
    selC8 = np.zeros((8, 128), np.float32)
    for k in range(4):
        selC8[4 + k, 32 * k:32 * k + 32] = wlc
    consts["selC8"] = selC8.astype(BF)

    blcol = np.zeros((128, 1), np.float32)
    WvP = np.zeros((128, 4), np.float32)
    for gl in range(4):
        blcol[32 * gl:32 * gl + 32, 0] = bl
        WvP[32 * gl:32 * gl + 32, gl] = Wv[:, 0]
    consts["blc"] = blcol
    consts["WvP"] = WvP
    consts["WvPb"] = WvP.astype(BF)

    in_maps = []
    for t in range(NCORES):
        m = dict(consts)
        xs = x[t * GPC:(t + 1) * GPC]            # [128, 64, 4]
        eas = ea[t * GPC:(t + 1) * GPC]          # [128, 1024, 2]
        acs = act[t * GPC:(t + 1) * GPC]         # [128, 1027]

        gxs = xs[:, es, :]                       # [128, 1024, 4]
        gxd = xs[:, ed, :]
        rows = np.concatenate([gxs.transpose(0, 2, 1),
                               gxd.transpose(0, 2, 1)], axis=1)  # [128, 8, 1024]
        gxE = np.empty((81, 16 * NE), np.float32)
        gxE[0:64] = rows.reshape(16, 64, NE).transpose(1, 0, 2).reshape(64, 16 * NE)
        erows = eas.transpose(0, 2, 1).reshape(16, 16, NE)
        gxE[64:80] = erows.transpose(1, 0, 2).reshape(16, 16 * NE)
        gxE[80] = 1.0
        m["gxE"] = np.ascontiguousarray(gxE).astype(BF)

        xT2c = np.empty((17, 32 * 64), np.float32)
        xt = xs.reshape(32, 4, NN, NODE).transpose(0, 1, 3, 2)
        xT2c[0:16] = xt.reshape(32, 16, NN).transpose(1, 0, 2).reshape(16, 32 * NN)
        xT2c[16] = np.tile(cn, 32)
        m["xT2c"] = xT2c.astype(BF)

        a4 = acs.reshape(32, 4, E2)
        actH0 = np.concatenate([
            a4[0::2].transpose(1, 0, 2).reshape(4, 16 * E2),
            a4[1::2].transpose(1, 0, 2).reshape(4, 16 * E2)], axis=0)
        m["actH0"] = np.ascontiguousarray(actH0).astype(BF)
        m["actH1"] = np.ascontiguousarray(
            a4[1::2].transpose(1, 0, 2).reshape(4, 16 * E2)).astype(BF)
        in_maps.append(m)

    extra = float(E2) * float(bv.reshape(-1)[0])
    return in_maps, extra


def _assemble(results, extra):
    out = np.empty((B,), np.float32)
    for t in range(NCORES):
        v = results[t]["v"]                      # [4, 160]
        per = v[:, 0:64:2] + v[:, 1:64:2]        # [4, 32]
        facts = v[:, 64:160].reshape(4, 32, 3).sum(-1)
        out[t * GPC:(t + 1) * GPC] = (per + facts).T.reshape(-1) + extra
    return out


def kernel(**inputs) -> np.ndarray:
    if "nc" not in _CACHE:
        _CACHE["nc"] = _build_nc()
    nc = _CACHE["nc"]
    in_maps, extra = _host_prep(inputs)
    res = bass_utils.run_bass_kernel_spmd(nc, in_maps, list(range(NCORES)))
    return _assemble(res.results, extra)
